# revision 5
# baseline (speedup 1.0000x reference)
"""MoE graph projector (top-2 of 4 experts) on 8 Trainium2 NeuronCores.

Problem:
  router: logits = routing_features @ gate_w.T -> top-2 softmax weights
  combined[b] = sum_k w_k * (graph_emb[b] @ W_{e_k}.T + b_{e_k}), masked
  aux_loss: load-balance scalar (computed on host; tiny)

Sharding: 8-way split of D_LLM (each core computes a 512-column slice of the
output for ALL 256 graphs). Each core holds all 4 experts' weight slices
SBUF-resident in bf16 (9.96 MB, loaded once).

Key structure ("class pairing"): graphs are grouped by their unordered top-2
expert set (6 classes for E=4, K=2) and paired within a class. A pair (g, h)
shares both selected experts, so the PE computes with a full 128-wide
stationary operand:
  phase A: lhsT = [w_lo(g) * Xg^T, w_lo(h) * Xh^T]  (K=128 chunk, M=128 tokens)
           rhs  = W_{e_lo}^T chunk (streamed from SBUF)
  phase B: same with e_hi
  bias:    one K=2 matmul: lhsT = per-token gate weights [2,128],
           rhs = the two experts' bias rows [2,512]
All phases accumulate into one PSUM bank which then holds the FINAL combined
output slice for both graphs -> single eviction, no dispatch/combine pass.

The routing is known on the host before the device program is built, so the
per-pair expert choices are baked into the program as compile-time constants
(the compiled program is cached keyed on the expert schedule). Gate weights
and the graph mask are folded into X / the bias stationary on the host.
"""

import sys

if "/opt/trn_rl_repo" not in sys.path:
    sys.path.insert(0, "/opt/trn_rl_repo")

import numpy as np
import ml_dtypes

import concourse.bass as bass
import concourse.bacc as bacc
import concourse.tile as tile
import concourse.mybir as mybir
from concourse.bass_utils import run_bass_kernel_spmd

BF16 = mybir.dt.bfloat16
F32 = mybir.dt.float32
NP_BF16 = ml_dtypes.bfloat16

B, S, DMM, DLLM = 256, 64, 2432, 4096
E, K = 4, 2
N_CORES = 8
DQ = DLLM // N_CORES   # 512 output columns per core
KC = DMM // 128        # 19 contraction chunks
WSTRIDE = KC * DQ      # 9728 elements per expert in the SBUF weight tile
MAXP = B // 2 + 3      # 131: worst case adds 6 dummy slots (even # of odd classes)

# class id -> (e_lo, e_hi)
CLASSES = [(0, 1), (0, 2), (0, 3), (1, 2), (1, 3), (2, 3)]
CLASS_ID = {c: i for i, c in enumerate(CLASSES)}

_PROG_CACHE = {}


def build_program(pair_experts, enable_asserts=False):
    """Build the SPMD per-core Bass program for a concrete pair schedule.

    pair_experts: tuple of (e_lo, e_hi) per pair — compile-time constants.
    """
    key = (tuple(pair_experts), enable_asserts)
    if key in _PROG_CACHE:
        return _PROG_CACHE[key]

    T = len(pair_experts)
    nc = bacc.Bacc(
        "TRN2",
        target_bir_lowering=False,
        debug=False,
        enable_asserts=enable_asserts,
        num_devices=N_CORES,
    )
    xd = nc.dram_tensor("xd", [T, 2, 128, DMM], BF16, kind="ExternalInput").ap()
    wq = nc.dram_tensor("wq", [128, E * WSTRIDE], BF16, kind="ExternalInput").ap()
    wb = nc.dram_tensor("wb", [T, 2, 128 + DQ], BF16, kind="ExternalInput").ap()
    od = nc.dram_tensor("od", [T, 128, DQ], F32, kind="ExternalOutput").ap()

    with tile.TileContext(nc) as tc:
        with (
            tc.tile_pool(name="wpool", bufs=1) as wpool,
            tc.tile_pool(name="xpool", bufs=6) as xpool,
            tc.tile_pool(name="wbpool", bufs=3) as wbpool,
            tc.tile_pool(name="opool", bufs=3) as opool,
            tc.tile_pool(name="pspool", bufs=8, space="PSUM") as pspool,
        ):
            w_sb = wpool.tile([128, E * WSTRIDE], BF16)
            for e in range(E):
                nc.sync.dma_start(
                    w_sb[:, e * WSTRIDE : (e + 1) * WSTRIDE],
                    wq[:, e * WSTRIDE : (e + 1) * WSTRIDE],
                )

            for p in range(T):
                e_lo, e_hi = pair_experts[p]
                xts = []
                for ph in range(2):
                    xt = xpool.tile([128, DMM], BF16, tag="x", name=f"x_{p}_{ph}")
                    nc.sync.dma_start(xt[:], xd[p, ph])
                    xts.append(xt)
                wbt = wbpool.tile([2, 128 + DQ], BF16, tag="wb", name=f"wb_{p}")
                nc.sync.dma_start(wbt[:], wb[p])

                ps = pspool.tile([128, DQ], F32, tag="ps", name=f"ps_{p}")
                for ph, e in ((0, e_lo), (1, e_hi)):
                    for kc in range(KC):
                        nc.tensor.matmul(
                            ps[:],
                            xts[ph][:, kc * 128 : (kc + 1) * 128],
                            w_sb[:, e * WSTRIDE + kc * DQ : e * WSTRIDE + (kc + 1) * DQ],
                            start=(ph == 0 and kc == 0),
                            stop=False,
                        )
                # bias: out[t, d] += w_lo[t]*b_lo[d] + w_hi[t]*b_hi[d]
                nc.tensor.matmul(
                    ps[:],
                    wbt[:2, 0:128],
                    wbt[:2, 128 : 128 + DQ],
                    start=False,
                    stop=True,
                )
                ot = opool.tile([128, DQ], F32, tag="o", name=f"o_{p}")
                nc.vector.tensor_copy(out=ot[:], in_=ps[:])
                nc.sync.dma_start(od[p], ot[:])

    nc.compile()
    _PROG_CACHE[key] = nc
    return nc


def _softmax(x, axis=-1):
    x = x - x.max(axis=axis, keepdims=True)
    e = np.exp(x)
    return e / e.sum(axis=axis, keepdims=True)


def _router_host(routing_features, gate_w):
    """Router + aux loss. Uses jax on CPU when available so logits / top-k /
    aux match the f32 reference bit-for-bit; falls back to numpy."""
    try:
        import jax
        import jax.numpy as jnp

        cpu = jax.devices("cpu")[0]
        with jax.default_device(cpu):
            rf = jnp.asarray(routing_features)
            gw = jnp.asarray(gate_w)
            logits = rf @ gw.T
            full_probs = jax.nn.softmax(logits, axis=-1)
            top_logits, top_idx = jax.lax.top_k(logits, K)
            weights = jax.nn.softmax(top_logits, axis=-1)
            one_hot = jax.nn.one_hot(top_idx, E, dtype=jnp.float32)
            tokens_per_expert = one_hot.sum(axis=(0, 1))
            f = tokens_per_expert / (logits.shape[0] * K)
            P = full_probs.mean(axis=0)
            aux = E * jnp.sum(f * P)
        return np.asarray(top_idx), np.asarray(weights), np.asarray(aux)
    except Exception:
        logits = routing_features.astype(np.float32) @ gate_w.astype(np.float32).T
        top_idx = np.argsort(-logits, axis=1, kind="stable")[:, :K]
        top_logits = np.take_along_axis(logits, top_idx, axis=1)
        weights = _softmax(top_logits, axis=-1)
        counts = np.zeros(E, np.float32)
        np.add.at(counts, top_idx.reshape(-1), 1.0)
        f = counts / np.float32(B * K)
        P = _softmax(logits, axis=-1).mean(axis=0)
        aux = np.float32(E * np.sum(f * P))
        return top_idx, weights, aux


def _host_prep(graph_emb, routing_features, gate_w, expert_w, expert_b, graph_mask):
    """Builds the pair schedule, per-core input maps, and assembly metadata."""
    top_idx, top_w, aux = _router_host(routing_features, gate_w)

    mask_f = graph_mask.astype(np.float32)  # [B, S]

    # --- pair schedule (class-major) ---
    keys = np.sort(top_idx, axis=1)  # [B, 2]
    cls = np.array([CLASS_ID[(int(a), int(b))] for a, b in keys])
    pairs = []  # (g_a, g_b) with -1 dummies
    pair_experts = []
    for c in range(len(CLASSES)):
        members = list(np.nonzero(cls == c)[0])
        if len(members) % 2:
            members.append(-1)
        for j in range(0, len(members), 2):
            pairs.append((members[j], members[j + 1]))
            pair_experts.append(CLASSES[c])
    T = len(pairs)
    assert T <= MAXP

    slot_g = np.array([g for (a, b) in pairs for g in (a, b)], dtype=np.int64)
    e_lo = np.array([e for (e, _) in pair_experts])
    e_hi = np.array([e for (_, e) in pair_experts])

    valid = slot_g >= 0
    sg = slot_g[valid]

    # per-slot gate weight for the class's lo/hi expert
    w_slot = np.zeros((2 * T, 2), np.float32)  # [:, 0]=lo, [:, 1]=hi
    pe_lo = np.repeat(e_lo, 2)[valid]
    pe_hi = np.repeat(e_hi, 2)[valid]
    ti = top_idx[sg]
    tw = top_w[sg]
    w_slot[valid, 0] = np.where(ti[:, 0] == pe_lo, tw[:, 0], tw[:, 1])
    w_slot[valid, 1] = np.where(ti[:, 0] == pe_hi, tw[:, 0], tw[:, 1])

    # gather + mask embeddings [2T, S, DMM]
    emb_sel = np.zeros((2 * T, S, DMM), np.float32)
    emb_sel[valid] = graph_emb[sg] * mask_f[sg][:, :, None]

    # xd[p, ph, r, kc*128 + t] = w_slot * emb_sel[2p + t//64, t%64, kc*128+r]
    xd = np.empty((T, 2, 128, DMM), NP_BF16)
    for ph in range(2):
        scaled = (emb_sel * w_slot[:, ph][:, None, None]).astype(NP_BF16)
        t = scaled.reshape(T, 128, KC, 128).transpose(0, 3, 2, 1)
        xd[:, ph] = np.ascontiguousarray(t).reshape(T, 128, DMM)

    # per-token gate-weight stationary for the bias matmul (mask-folded)
    mask_sel = np.zeros((2 * T, S), np.float32)
    mask_sel[valid] = mask_f[sg]
    w2 = np.zeros((T, 2, 128), np.float32)
    for ph in range(2):
        w2[:, ph] = (w_slot[:, ph][:, None] * mask_sel).reshape(T, 128)
    w2 = w2.astype(NP_BF16)

    # weights arranged per core slice: wq[r, e*WSTRIDE + kc*DQ + d] =
    #   expert_w[e, q*DQ + d, kc*128 + r]
    w_bf = np.ascontiguousarray(expert_w.astype(NP_BF16))
    wr = w_bf.reshape(E, N_CORES, DQ, KC, 128)  # [e, q, d, kc, r]
    bias_bf = expert_b.astype(NP_BF16)  # [E, DLLM]

    in_maps = []
    for q in range(N_CORES):
        wq_q = np.ascontiguousarray(wr[:, q].transpose(3, 0, 2, 1)).reshape(
            128, E * WSTRIDE
        )
        wbq = np.zeros((T, 2, 128 + DQ), NP_BF16)
        wbq[:, :, :128] = w2
        wbq[:, 0, 128:] = bias_bf[e_lo, q * DQ : (q + 1) * DQ]
        wbq[:, 1, 128:] = bias_bf[e_hi, q * DQ : (q + 1) * DQ]
        in_maps.append({"xd": xd, "wq": wq_q, "wb": wbq})

    return in_maps, tuple(pair_experts), slot_g, aux


def _assemble(results, slot_g):
    T2 = len(slot_g)
    combined = np.zeros((B, S, DLLM), np.float32)
    valid = slot_g >= 0
    sg = slot_g[valid]
    for q in range(N_CORES):
        od = results[q]["od"].reshape(T2, S, DQ)
        combined[sg, :, q * DQ : (q + 1) * DQ] = od[valid]
    return combined


def kernel(graph_emb, routing_features, gate_w, expert_w, expert_b, graph_mask):
    in_maps, pair_experts, slot_g, aux = _host_prep(
        graph_emb, routing_features, gate_w, expert_w, expert_b, graph_mask
    )
    nc = build_program(pair_experts)
    res = run_bass_kernel_spmd(nc, in_maps, core_ids=list(range(N_CORES)))
    combined = _assemble(res.results, slot_g)
    return combined, aux


# revision 8
# speedup vs baseline: 1.0551x; 1.0551x over previous
"""MoE graph projector (top-2 of 4 experts) on 8 Trainium2 NeuronCores.

Problem:
  router: logits = routing_features @ gate_w.T -> top-2 softmax weights
  combined[b] = sum_k w_k * (graph_emb[b] @ W_{e_k}.T + b_{e_k}), masked
  aux_loss: load-balance scalar (computed on host; tiny)

Sharding: 8-way split of D_LLM (each core computes a 512-column slice of the
output for ALL 256 graphs). Each core holds all 4 experts' weight slices
SBUF-resident in bf16 (9.96 MB, loaded once).

Key structure ("class pairing"): graphs are grouped by their unordered top-2
expert set (6 classes for E=4, K=2) and paired within a class. A pair (g, h)
shares both selected experts, so the PE computes with a full 128-wide
stationary operand:
  phase A: lhsT = [w_lo(g) * Xg^T, w_lo(h) * Xh^T]  (K=128 chunk, M=128 tokens)
           rhs  = W_{e_lo}^T chunk (streamed from SBUF)
  phase B: same with e_hi
Both phases accumulate into one PSUM bank which then holds the combined
output slice (sans bias) for both graphs. The eviction is a single DVE add
(psum + host-materialized per-pair bias tile) -> no dispatch/combine pass.

The routing is known on the host before the device program is built, so the
per-pair expert choices are baked into the program as compile-time constants
(the compiled program is cached keyed on the expert schedule). Gate weights
and the graph mask are folded into X / the bias stationary on the host.
"""

import sys

if "/opt/trn_rl_repo" not in sys.path:
    sys.path.insert(0, "/opt/trn_rl_repo")

import numpy as np
import ml_dtypes

import concourse.bass as bass
import concourse.bacc as bacc
import concourse.tile as tile
import concourse.mybir as mybir
from concourse.bass_utils import run_bass_kernel_spmd

BF16 = mybir.dt.bfloat16
F32 = mybir.dt.float32
NP_BF16 = ml_dtypes.bfloat16

B, S, DMM, DLLM = 256, 64, 2432, 4096
E, K = 4, 2
N_CORES = 8
DQ = DLLM // N_CORES   # 512 output columns per core
KC = DMM // 128        # 19 contraction chunks
WSTRIDE = KC * DQ      # 9728 elements per expert in the SBUF weight tile
MAXP = B // 2 + 3      # 131: worst case adds 6 dummy slots (even # of odd classes)

# class id -> (e_lo, e_hi)
CLASSES = [(0, 1), (0, 2), (0, 3), (1, 2), (1, 3), (2, 3)]
CLASS_ID = {c: i for i, c in enumerate(CLASSES)}

_PROG_CACHE = {}


def build_program(pair_experts, enable_asserts=False):
    """Build the SPMD per-core Bass program for a concrete pair schedule.

    pair_experts: tuple of (e_lo, e_hi) per pair — compile-time constants.
    """
    key = (tuple(pair_experts), enable_asserts)
    if key in _PROG_CACHE:
        return _PROG_CACHE[key]

    T = len(pair_experts)
    nc = bacc.Bacc(
        "TRN2",
        target_bir_lowering=False,
        debug=False,
        enable_asserts=enable_asserts,
        num_devices=N_CORES,
    )
    xd = nc.dram_tensor("xd", [T, 2, 128, DMM], BF16, kind="ExternalInput").ap()
    wq = nc.dram_tensor("wq", [128, E * WSTRIDE], BF16, kind="ExternalInput").ap()
    bd = nc.dram_tensor("bd", [T, 128, DQ], BF16, kind="ExternalInput").ap()
    od = nc.dram_tensor("od", [T, 128, DQ], F32, kind="ExternalOutput").ap()

    with tile.TileContext(nc) as tc:
        with (
            tc.tile_pool(name="wpool", bufs=1) as wpool,
            tc.tile_pool(name="xpool", bufs=8) as xpool,
            tc.tile_pool(name="bdpool", bufs=4) as bdpool,
            tc.tile_pool(name="opool", bufs=3) as opool,
            tc.tile_pool(name="pspool", bufs=8, space="PSUM") as pspool,
        ):
            # Prefetch the first pairs' inputs ahead of the big weight DMAs so
            # the PE can start as soon as the first expert slice lands.
            n_pre = min(3, T)
            pre_x = {}
            pre_bd = {}
            for p in range(n_pre):
                xts = []
                for ph in range(2):
                    xt = xpool.tile([128, DMM], BF16, tag="x", name=f"x_{p}_{ph}")
                    nc.sync.dma_start(xt[:], xd[p, ph])
                    xts.append(xt)
                pre_x[p] = xts
                bdt = bdpool.tile([128, DQ], BF16, tag="bd", name=f"bd_{p}")
                nc.sync.dma_start(bdt[:], bd[p])
                pre_bd[p] = bdt

            # Weight slices in first-use order.
            w_sb = wpool.tile([128, E * WSTRIDE], BF16)
            e_order = []
            for e_lo, e_hi in pair_experts:
                for e in (e_lo, e_hi):
                    if e not in e_order:
                        e_order.append(e)
            for e in range(E):
                if e not in e_order:
                    e_order.append(e)
            for e in e_order:
                nc.sync.dma_start(
                    w_sb[:, e * WSTRIDE : (e + 1) * WSTRIDE],
                    wq[:, e * WSTRIDE : (e + 1) * WSTRIDE],
                )

            for p in range(T):
                e_lo, e_hi = pair_experts[p]
                if p in pre_x:
                    xts = pre_x.pop(p)
                    bdt = pre_bd.pop(p)
                else:
                    xts = []
                    for ph in range(2):
                        xt = xpool.tile([128, DMM], BF16, tag="x", name=f"x_{p}_{ph}")
                        nc.sync.dma_start(xt[:], xd[p, ph])
                        xts.append(xt)
                    bdt = bdpool.tile([128, DQ], BF16, tag="bd", name=f"bd_{p}")
                    nc.sync.dma_start(bdt[:], bd[p])

                ps = pspool.tile([128, DQ], F32, tag="ps", name=f"ps_{p}")
                for ph, e in ((0, e_lo), (1, e_hi)):
                    for kc in range(KC):
                        nc.tensor.matmul(
                            ps[:],
                            xts[ph][:, kc * 128 : (kc + 1) * 128],
                            w_sb[:, e * WSTRIDE + kc * DQ : e * WSTRIDE + (kc + 1) * DQ],
                            start=(ph == 0 and kc == 0),
                            stop=(ph == 1 and kc == KC - 1),
                        )
                # eviction folds in the bias: out = psum + bias_tile
                ot = opool.tile([128, DQ], F32, tag="o", name=f"o_{p}")
                nc.vector.tensor_tensor(
                    out=ot[:], in0=ps[:], in1=bdt[:], op=mybir.AluOpType.add
                )
                nc.sync.dma_start(od[p], ot[:])

    nc.compile()
    _PROG_CACHE[key] = nc
    return nc


def _softmax(x, axis=-1):
    x = x - x.max(axis=axis, keepdims=True)
    e = np.exp(x)
    return e / e.sum(axis=axis, keepdims=True)


def _router_host(routing_features, gate_w):
    """Router + aux loss. Uses jax on CPU when available so logits / top-k /
    aux match the f32 reference bit-for-bit; falls back to numpy."""
    try:
        import jax
        import jax.numpy as jnp

        cpu = jax.devices("cpu")[0]
        with jax.default_device(cpu):
            rf = jnp.asarray(routing_features)
            gw = jnp.asarray(gate_w)
            logits = rf @ gw.T
            full_probs = jax.nn.softmax(logits, axis=-1)
            top_logits, top_idx = jax.lax.top_k(logits, K)
            weights = jax.nn.softmax(top_logits, axis=-1)
            one_hot = jax.nn.one_hot(top_idx, E, dtype=jnp.float32)
            tokens_per_expert = one_hot.sum(axis=(0, 1))
            f = tokens_per_expert / (logits.shape[0] * K)
            P = full_probs.mean(axis=0)
            aux = E * jnp.sum(f * P)
        return np.asarray(top_idx), np.asarray(weights), np.asarray(aux)
    except Exception:
        logits = routing_features.astype(np.float32) @ gate_w.astype(np.float32).T
        top_idx = np.argsort(-logits, axis=1, kind="stable")[:, :K]
        top_logits = np.take_along_axis(logits, top_idx, axis=1)
        weights = _softmax(top_logits, axis=-1)
        counts = np.zeros(E, np.float32)
        np.add.at(counts, top_idx.reshape(-1), 1.0)
        f = counts / np.float32(B * K)
        P = _softmax(logits, axis=-1).mean(axis=0)
        aux = np.float32(E * np.sum(f * P))
        return top_idx, weights, aux


def _host_prep(graph_emb, routing_features, gate_w, expert_w, expert_b, graph_mask):
    """Builds the pair schedule, per-core input maps, and assembly metadata."""
    top_idx, top_w, aux = _router_host(routing_features, gate_w)

    mask_f = graph_mask.astype(np.float32)  # [B, S]

    # --- pair schedule (class-major) ---
    keys = np.sort(top_idx, axis=1)  # [B, 2]
    cls = np.array([CLASS_ID[(int(a), int(b))] for a, b in keys])
    pairs = []  # (g_a, g_b) with -1 dummies
    pair_experts = []
    for c in range(len(CLASSES)):
        members = list(np.nonzero(cls == c)[0])
        if len(members) % 2:
            members.append(-1)
        for j in range(0, len(members), 2):
            pairs.append((members[j], members[j + 1]))
            pair_experts.append(CLASSES[c])
    T = len(pairs)
    assert T <= MAXP

    slot_g = np.array([g for (a, b) in pairs for g in (a, b)], dtype=np.int64)
    e_lo = np.array([e for (e, _) in pair_experts])
    e_hi = np.array([e for (_, e) in pair_experts])

    valid = slot_g >= 0
    sg = slot_g[valid]

    # per-slot gate weight for the class's lo/hi expert
    w_slot = np.zeros((2 * T, 2), np.float32)  # [:, 0]=lo, [:, 1]=hi
    pe_lo = np.repeat(e_lo, 2)[valid]
    pe_hi = np.repeat(e_hi, 2)[valid]
    ti = top_idx[sg]
    tw = top_w[sg]
    w_slot[valid, 0] = np.where(ti[:, 0] == pe_lo, tw[:, 0], tw[:, 1])
    w_slot[valid, 1] = np.where(ti[:, 0] == pe_hi, tw[:, 0], tw[:, 1])

    # gather + mask embeddings [2T, S, DMM]
    emb_sel = np.zeros((2 * T, S, DMM), np.float32)
    emb_sel[valid] = graph_emb[sg] * mask_f[sg][:, :, None]

    # xd[p, ph, r, kc*128 + t] = w_slot * emb_sel[2p + t//64, t%64, kc*128+r]
    xd = np.empty((T, 2, 128, DMM), NP_BF16)
    for ph in range(2):
        scaled = (emb_sel * w_slot[:, ph][:, None, None]).astype(NP_BF16)
        t = scaled.reshape(T, 128, KC, 128).transpose(0, 3, 2, 1)
        xd[:, ph] = np.ascontiguousarray(t).reshape(T, 128, DMM)

    # per-token effective gate weights (mask-folded) for the bias tiles
    mask_sel = np.zeros((2 * T, S), np.float32)
    mask_sel[valid] = mask_f[sg]
    w2 = np.zeros((T, 2, 128), np.float32)
    for ph in range(2):
        w2[:, ph] = (w_slot[:, ph][:, None] * mask_sel).reshape(T, 128)

    # weights arranged per core slice: wq[r, e*WSTRIDE + kc*DQ + d] =
    #   expert_w[e, q*DQ + d, kc*128 + r]
    w_bf = np.ascontiguousarray(expert_w.astype(NP_BF16))
    wr = w_bf.reshape(E, N_CORES, DQ, KC, 128)  # [e, q, d, kc, r]
    bias_f = expert_b.astype(np.float32)  # [E, DLLM]
    blo = bias_f[e_lo]  # [T, DLLM]
    bhi = bias_f[e_hi]

    in_maps = []
    for q in range(N_CORES):
        qs = slice(q * DQ, (q + 1) * DQ)
        wq_q = np.ascontiguousarray(wr[:, q].transpose(3, 0, 2, 1)).reshape(
            128, E * WSTRIDE
        )
        # bd[p, t, d] = w_lo[t]*b_lo[d] + w_hi[t]*b_hi[d]  (mask folded into w)
        bd_q = (
            w2[:, 0, :, None] * blo[:, None, qs] + w2[:, 1, :, None] * bhi[:, None, qs]
        ).astype(NP_BF16)
        in_maps.append({"xd": xd, "wq": wq_q, "bd": bd_q})

    return in_maps, tuple(pair_experts), slot_g, aux


def _assemble(results, slot_g):
    T2 = len(slot_g)
    combined = np.zeros((B, S, DLLM), np.float32)
    valid = slot_g >= 0
    sg = slot_g[valid]
    for q in range(N_CORES):
        od = results[q]["od"].reshape(T2, S, DQ)
        combined[sg, :, q * DQ : (q + 1) * DQ] = od[valid]
    return combined


def kernel(graph_emb, routing_features, gate_w, expert_w, expert_b, graph_mask):
    in_maps, pair_experts, slot_g, aux = _host_prep(
        graph_emb, routing_features, gate_w, expert_w, expert_b, graph_mask
    )
    nc = build_program(pair_experts)
    res = run_bass_kernel_spmd(nc, in_maps, core_ids=list(range(N_CORES)))
    combined = _assemble(res.results, slot_g)
    return combined, aux
